# revision 21
# baseline (speedup 1.0000x reference)
"""Trainium2 Bass kernel for the 2-hop key-value memory network.

Strategy: data-parallel over batch (B=32 -> 4 per core x 8 cores).

The gather descriptor generation on the GPSIMD (SWDGE Q7) engine is the
hard bottleneck (~8.4 ns per gathered row, serialized), so this version
minimizes gathered rows and hides all other work under the gather stream:

  - Host builds a PER-CORE dense table: the ~31k unique tokens a core
    touches, remapped to dense ids < 32768.  This kills the int16 lo/hi
    index split and its ~25% slot padding: exactly 48 gathers x 1024 rows
    per core, streamed in natural (mem, word) order.
  - Natural order makes the word-sum selection matrix a fixed constant
    (slot s -> mem s//8), so no per-call selection-matrix DMA.
  - Loops are ordered b-outer so each local batch's hop math (scores,
    renorms, weighted sums, final bilinear form) runs under the next
    batch's gathers; only the last batch's hop chain is exposed.

The log-softmax renorms reduce to per-row affine transforms:
  renorm_q: p = (s - min(s)) / (sum(s) - 512*min(s) + 512e-8)
  renorm_a: p = (s - lse)   / (512*lse - sum(s)),  lse = logsumexp(s)
"""
import sys

for _p in ("/opt/pypackages", "/opt/trn_rl_repo"):
    if _p not in sys.path:
        sys.path.insert(0, _p)

import numpy as np
import ml_dtypes

import concourse.bass as bass
import concourse.bacc as bacc
import concourse.mybir as mybir
import concourse.tile as tile
from concourse.bass_utils import run_bass_kernel_spmd

BF = ml_dtypes.bfloat16

# problem constants
B, NMEM, NW, QLEN, NCH, CLEN = 32, 512, 8, 32, 8, 8
VOCAB, M = 50000, 256
EMB = 3 * M           # 768
NCORES = 8
BL = B // NCORES      # 4 batch per core
NCHUNK = 4            # mem chunks of 128 per (b, tau)
BU = 384              # b-table unique slots per core

_cache = {}


def _wrap_idx16(stream):
    """stream (len multiple of 16) -> [128, len/16] int16 wrapped layout:
    index i lives at [i % 16, i // 16], replicated across the 8 groups of 16
    partitions."""
    c = len(stream) // 16
    arr = stream.astype(np.uint16).reshape(c, 16).T
    return np.tile(arr, (8, 1)).view(np.int16)


def _renorm_rows(nc, hsb, ppool, S, rows, maskq, hop, b):
    """Affine renorm of 9 score rows: P = (S - A) / B with
    q-row (p%32==0): A=min, B=sum-512*min+512e-8
    a-rows:          A=lse, B=512*lse-sum."""
    f32 = mybir.dt.float32
    bf16 = mybir.dt.bfloat16
    t = lambda tag: hsb.tile([128, 1], f32, tag=tag, name=tag)
    mx, mn, sm = t("mx"), t("mn"), t("sm")
    nc.vector.tensor_reduce(out=mx[rows], in_=S[rows],
                            axis=mybir.AxisListType.X, op=mybir.AluOpType.max)
    nc.vector.tensor_reduce(out=mn[rows], in_=S[rows],
                            axis=mybir.AxisListType.X, op=mybir.AluOpType.min)
    nc.vector.tensor_reduce(out=sm[rows], in_=S[rows],
                            axis=mybir.AxisListType.X, op=mybir.AluOpType.add)
    negmx = t("negmx")
    nc.vector.tensor_scalar(out=negmx[rows], in0=mx[rows], scalar1=-1.0,
                            scalar2=None, op0=mybir.AluOpType.mult)
    texp = ppool.tile([128, NMEM], f32, tag="texp")
    se = t("se")
    nc.scalar.activation(out=texp[rows], in_=S[rows],
                         func=mybir.ActivationFunctionType.Exp,
                         bias=negmx[rows], scale=1.0, accum_out=se[rows])
    lse = t("lse")
    nc.scalar.activation(out=lse[rows], in_=se[rows],
                         func=mybir.ActivationFunctionType.Ln)
    nc.vector.tensor_tensor(out=lse[rows], in0=lse[rows], in1=mx[rows],
                            op=mybir.AluOpType.add)
    t1 = t("t1")
    nc.vector.tensor_tensor(out=t1[rows], in0=mn[rows], in1=lse[rows],
                            op=mybir.AluOpType.subtract)
    nc.vector.tensor_tensor(out=t1[rows], in0=t1[rows], in1=maskq[rows],
                            op=mybir.AluOpType.mult)
    Av = t("Av")
    nc.vector.tensor_tensor(out=Av[rows], in0=lse[rows], in1=t1[rows],
                            op=mybir.AluOpType.add)
    bq = t("bq")
    nc.vector.tensor_scalar(out=bq[rows], in0=mn[rows], scalar1=-512.0,
                            scalar2=512e-8, op0=mybir.AluOpType.mult,
                            op1=mybir.AluOpType.add)
    nc.vector.tensor_tensor(out=bq[rows], in0=bq[rows], in1=sm[rows],
                            op=mybir.AluOpType.add)
    ba = t("ba")
    nc.vector.tensor_scalar(out=ba[rows], in0=lse[rows], scalar1=512.0,
                            scalar2=None, op0=mybir.AluOpType.mult)
    nc.vector.tensor_tensor(out=ba[rows], in0=ba[rows], in1=sm[rows],
                            op=mybir.AluOpType.subtract)
    Bv = t("Bv")
    nc.vector.tensor_tensor(out=Bv[rows], in0=bq[rows], in1=ba[rows],
                            op=mybir.AluOpType.subtract)
    nc.vector.tensor_tensor(out=Bv[rows], in0=Bv[rows], in1=maskq[rows],
                            op=mybir.AluOpType.mult)
    nc.vector.tensor_tensor(out=Bv[rows], in0=Bv[rows], in1=ba[rows],
                            op=mybir.AluOpType.add)
    invb = t("invb")
    nc.vector.reciprocal(out=invb[rows], in_=Bv[rows])
    P = ppool.tile([128, NMEM], bf16, tag="P")
    nc.vector.tensor_scalar(out=P[rows], in0=S[rows], scalar1=Av[rows],
                            scalar2=invb[rows], op0=mybir.AluOpType.subtract,
                            op1=mybir.AluOpType.mult)
    return P


def _build_program(nu_pad):
    """Build + compile the SPMD program (same for all cores)."""
    if nu_pad in _cache:
        return _cache[nu_pad]

    f32 = mybir.dt.float32
    bf16 = mybir.dt.bfloat16

    nc = bacc.Bacc("TRN2", target_bir_lowering=False, debug=False,
                   num_swdge_queues=2,
                   dynamic_dma_scratch_size=24576)
    tab_d = nc.dram_tensor("tab", [nu_pad, EMB], bf16, kind="ExternalInput")
    btab_d = nc.dram_tensor("btab", [BU, EMB], bf16, kind="ExternalInput")
    ut_d = nc.dram_tensor("ut", [EMB, EMB], bf16, kind="ExternalInput")
    vt_d = nc.dram_tensor("vt", [EMB, EMB], bf16, kind="ExternalInput")
    w_d = nc.dram_tensor("w", [EMB, EMB], bf16, kind="ExternalInput")
    idx16_d = nc.dram_tensor("idx16", [128, 3 * BL, NCHUNK * 64],
                             mybir.dt.int16, kind="ExternalInput")
    sel_d = nc.dram_tensor("sel", [128, 8 * 128], bf16, kind="ExternalInput")
    ones3_d = nc.dram_tensor("ones3", [3, 128, 128], bf16,
                             kind="ExternalInput")
    maskq_d = nc.dram_tensor("maskq", [128, 1], f32, kind="ExternalInput")
    identb_d = nc.dram_tensor("identb", [128, 128], bf16,
                              kind="ExternalInput")
    identf_d = nc.dram_tensor("identf", [128, 128], f32,
                              kind="ExternalInput")
    idxua_d = nc.dram_tensor("idxua", [128, 3], mybir.dt.int32,
                             kind="ExternalInput")
    out_d = nc.dram_tensor("pred", [BL, NCH], f32, kind="ExternalOutput")

    with tile.TileContext(nc) as tc:
        with (
            tc.tile_pool(name="const", bufs=1) as constp,
            tc.tile_pool(name="state", bufs=1) as statep,
            tc.tile_pool(name="gp", bufs=7) as gp,
            tc.tile_pool(name="ep", bufs=2) as ep,
            tc.tile_pool(name="stgp", bufs=2) as stgp,
            tc.tile_pool(name="hsb", bufs=1) as hsb,
            tc.tile_pool(name="ppool", bufs=2) as ppool,
            tc.tile_pool(name="wsps", bufs=1, space="PSUM") as wsps,
            tc.tile_pool(name="tpps", bufs=2, space="PSUM") as tpps,
            tc.tile_pool(name="hps", bufs=1, space="PSUM") as hps,
            tc.tile_pool(name="hps2", bufs=2, space="PSUM") as hps2,
        ):
            # ---------------- constants ----------------
            idx_sb = constp.tile([128, 3 * BL, NCHUNK * 64], mybir.dt.int16,
                                 tag="idx")
            nc.sync.dma_start(out=idx_sb[:], in_=idx16_d[:])
            sel_sb = constp.tile([128, 8 * 128], bf16, tag="sel")
            nc.sync.dma_start(out=sel_sb[:], in_=sel_d[:])
            identb = constp.tile([128, 128], bf16, tag="identb")
            nc.sync.dma_start(out=identb[:], in_=identb_d[:])
            identf = constp.tile([128, 128], f32, tag="identf")
            nc.sync.dma_start(out=identf[:], in_=identf_d[:])
            maskq = constp.tile([128, 1], f32, tag="maskq")
            nc.sync.dma_start(out=maskq[:], in_=maskq_d[:])
            ones3 = [constp.tile([128, 128], bf16, tag=f"ones{i}",
                                 name=f"ones{i}") for i in range(3)]
            for i in range(3):
                nc.sync.dma_start(out=ones3[i][:], in_=ones3_d[i])
            idxua = constp.tile([128, 3], mybir.dt.int32, tag="idxua")
            nc.sync.dma_start(out=idxua[:], in_=idxua_d[:])
            ut_sb = constp.tile([128, 6, EMB], bf16, tag="ut")
            vt_sb = constp.tile([128, 6, EMB], bf16, tag="vt")
            w_sb = constp.tile([128, 6, EMB], bf16, tag="w")

            def emit_uvw_dma():
                for t_sb, t_d in ((ut_sb, ut_d), (vt_sb, vt_d),
                                  (w_sb, w_d)):
                    nc.sync.dma_start(
                        out=t_sb[:],
                        in_=t_d[:].rearrange("(j p) d -> p j d", p=128))

            # persistent state
            X = statep.tile([128, EMB], f32, tag="X")
            o_sb = statep.tile([128, EMB], bf16, tag="o_sb")
            xtq = statep.tile([128, 6, BL], bf16, tag="xtq")
            xta = statep.tile([128, 6, 8 * BL], bf16, tag="xta")
            ysb0 = statep.tile([128, 6, 9 * BL], bf16, tag="ysb0")
            pred_sb = statep.tile([128, NCH], f32, tag="pred_sb")
            gb = statep.tile([128, 3, EMB], bf16, tag="gb")

            # ---------------- init: u and a from B_table ----------------
            # emitted inside the b-loop (after the first chunk's gathers) so
            # the gather stream starts immediately
            def emit_init():
                for i in range(3):
                    nc.gpsimd.indirect_dma_start(
                        out=gb[:, i, :], out_offset=None, in_=btab_d[:],
                        in_offset=bass.IndirectOffsetOnAxis(
                            ap=idxua[:, i:i + 1], axis=0))
                psi = hps.tile([128, EMB], f32, tag="O", name="psi")
                for i in range(3):
                    nc.tensor.matmul(out=psi[:, 0:512], lhsT=ones3[i][:],
                                     rhs=gb[:, i, 0:512],
                                     start=(i == 0), stop=(i == 2))
                for i in range(3):
                    nc.tensor.matmul(out=psi[:, 512:768], lhsT=ones3[i][:],
                                     rhs=gb[:, i, 512:768],
                                     start=(i == 0), stop=(i == 2))
                nc.vector.tensor_copy(out=X[:, 0:512], in_=psi[:, 0:512])
                nc.vector.tensor_copy(out=X[:, 512:768], in_=psi[:, 512:768])
                for j in range(6):
                    tp = tpps.tile([128, 128], f32, tag="tp", name="tpi")
                    nc.tensor.transpose(out=tp[:],
                                        in_=X[:, 128 * j:128 * j + 128],
                                        identity=identf[:])
                    tpv = tp[:].rearrange("p (b n) -> p b n", b=BL)
                    nc.any.tensor_copy(out=xtq[:, j, :], in_=tpv[:, :, 0])
                    nc.any.tensor_copy(out=xta[:, j, :], in_=tpv[:, :, 1:9])
                for i in range(6):
                    y0 = hps.tile([128, EMB], f32, tag="O", name="y0")
                    for j in range(6):
                        nc.tensor.matmul(
                            out=y0[:, 0:BL],
                            lhsT=ut_sb[:, j, 128 * i:128 * i + 128],
                            rhs=xtq[:, j, :], start=(j == 0), stop=(j == 5))
                    for j in range(6):
                        nc.tensor.matmul(
                            out=y0[:, BL:9 * BL],
                            lhsT=vt_sb[:, j, 128 * i:128 * i + 128],
                            rhs=xta[:, j, :], start=(j == 0), stop=(j == 5))
                    y9 = ysb0[:, i, :].rearrange("p (b r) -> p b r", b=BL)
                    ya4 = y0[:, BL:9 * BL].rearrange("p (b r) -> p b r", b=BL)
                    nc.any.tensor_copy(out=y9[:, :, 0], in_=y0[:, 0:BL])
                    nc.any.tensor_copy(out=y9[:, :, 1:9], in_=ya4[:])

            # ---------------- per-batch pipeline ----------------
            # hop math is emitted in segments interleaved between the NEXT
            # batch's chunks so PE waits (renorms) always overlap queued
            # word-sum matmuls.
            def hop_chain(b, S0, E1b, E2b, E1Tb):
                rows = slice(32 * b, 32 * b + 9)
                # --- segment A: hop0 renorm ---
                P0 = _renorm_rows(nc, hsb, ppool, S0[:], rows, maskq, 0, b)
                yield
                # --- segment B: hop0 weighted sum + state update ---
                PT = hsb.tile([128, NCHUNK, 16], bf16, tag=f"PT0_{b}",
                              name="PT")
                for k in range(NCHUNK):
                    tp = tpps.tile([128, 128], bf16, tag="tp", name="tp")
                    nc.tensor.transpose(
                        out=tp[:], in_=P0[:, 128 * k:128 * k + 128],
                        identity=identb[:])
                    nc.any.tensor_copy(out=PT[:, k, 0:9],
                                       in_=tp[:, 32 * b:32 * b + 9])
                O0 = hps.tile([128, EMB], f32, tag="O", name="O0")
                for k in range(NCHUNK):
                    nc.tensor.matmul(
                        out=O0[rows, 0:512], lhsT=PT[:, k, 0:9],
                        rhs=E1b[:, k, 0:512],
                        start=(k == 0), stop=(k == NCHUNK - 1),
                        tile_position=(0, 32 * b))
                    nc.tensor.matmul(
                        out=O0[rows, 512:768], lhsT=PT[:, k, 0:9],
                        rhs=E1b[:, k, 512:768],
                        start=(k == 0), stop=(k == NCHUNK - 1),
                        tile_position=(0, 32 * b))
                yield
                nc.vector.tensor_tensor(out=X[rows, 0:512],
                                        in0=X[rows, 0:512],
                                        in1=O0[rows, 0:512],
                                        op=mybir.AluOpType.add)
                nc.vector.tensor_tensor(out=X[rows, 512:768],
                                        in0=X[rows, 512:768],
                                        in1=O0[rows, 512:768],
                                        op=mybir.AluOpType.add)
                xt1 = hsb.tile([128, 6, 16], bf16, tag=f"xt1_{b}",
                               name="xt1")
                for j in range(6):
                    tp = tpps.tile([128, 128], f32, tag="tp", name="tp")
                    nc.tensor.transpose(
                        out=tp[:], in_=X[:, 128 * j:128 * j + 128],
                        identity=identf[:])
                    nc.any.tensor_copy(out=xt1[:, j, 0:9],
                                       in_=tp[:, 32 * b:32 * b + 9])
                yield
                # --- segment C: hop1 projections + scores + renorm ---
                ysb1 = hsb.tile([128, 6, 9], bf16, tag=f"ysb1_{b}",
                                name="ysb1")
                for i in range(6):
                    if i == 3:
                        yield
                    y1 = hps.tile([128, EMB], f32, tag="O", name="y1")
                    for j in range(6):
                        nc.tensor.matmul(
                            out=y1[:, 0:1],
                            lhsT=ut_sb[:, j, 128 * i:128 * i + 128],
                            rhs=xt1[:, j, 0:1], start=(j == 0),
                            stop=(j == 5))
                    for j in range(6):
                        nc.tensor.matmul(
                            out=y1[:, 1:9],
                            lhsT=vt_sb[:, j, 128 * i:128 * i + 128],
                            rhs=xt1[:, j, 1:9], start=(j == 0),
                            stop=(j == 5))
                    nc.any.tensor_copy(out=ysb1[:, i, :], in_=y1[:, 0:9])
                yield
                S1 = hps2.tile([128, NMEM], f32, tag="S", name="S1")
                for j in range(6):
                    nc.tensor.matmul(
                        out=S1[rows, :], lhsT=ysb1[:, j, 0:9],
                        rhs=E1Tb[:, j, :], start=(j == 0), stop=(j == 5),
                        tile_position=(0, 32 * b))
                P1 = _renorm_rows(nc, hsb, ppool, S1[:], rows, maskq, 1, b)
                yield
                # --- segment D: hop1 weighted sum + final bilinear form ---
                PT1 = hsb.tile([128, NCHUNK, 16], bf16, tag=f"PT1_{b}",
                               name="PT1")
                for k in range(NCHUNK):
                    tp = tpps.tile([128, 128], bf16, tag="tp", name="tp")
                    nc.tensor.transpose(
                        out=tp[:], in_=P1[:, 128 * k:128 * k + 128],
                        identity=identb[:])
                    nc.any.tensor_copy(out=PT1[:, k, 0:9],
                                       in_=tp[:, 32 * b:32 * b + 9])
                O1 = hps.tile([128, EMB], f32, tag="O", name="O1")
                for k in range(NCHUNK):
                    nc.tensor.matmul(
                        out=O1[rows, 0:512], lhsT=PT1[:, k, 0:9],
                        rhs=E2b[:, k, 0:512],
                        start=(k == 0), stop=(k == NCHUNK - 1),
                        tile_position=(0, 32 * b))
                    nc.tensor.matmul(
                        out=O1[rows, 512:768], lhsT=PT1[:, k, 0:9],
                        rhs=E2b[:, k, 512:768],
                        start=(k == 0), stop=(k == NCHUNK - 1),
                        tile_position=(0, 32 * b))
                yield
                nc.any.tensor_copy(out=o_sb[rows, 0:512],
                                   in_=O1[rows, 0:512])
                nc.any.tensor_copy(out=o_sb[rows, 512:768],
                                   in_=O1[rows, 512:768])
                ot = hsb.tile([128, 6, 16], bf16, tag=f"ot_{b}", name="ot")
                for j in range(6):
                    tp = tpps.tile([128, 128], bf16, tag="tp", name="tp")
                    nc.tensor.transpose(
                        out=tp[:], in_=o_sb[:, 128 * j:128 * j + 128],
                        identity=identb[:])
                    nc.any.tensor_copy(out=ot[:, j, 0:9],
                                       in_=tp[:, 32 * b:32 * b + 9])
                yield
                wq = hsb.tile([128, 6, 1], bf16, tag=f"wq_{b}", name="wq")
                for i in range(6):
                    wqp = hps.tile([128, EMB], f32, tag="O", name="wqp")
                    for j in range(6):
                        nc.tensor.matmul(
                            out=wqp[:, 0:1],
                            lhsT=w_sb[:, j, 128 * i:128 * i + 128],
                            rhs=ot[:, j, 0:1], start=(j == 0), stop=(j == 5))
                    nc.any.tensor_copy(out=wq[:, i, :], in_=wqp[:, 0:1])
                predp = hps.tile([128, EMB], f32, tag="O", name="predp")
                for i in range(6):
                    nc.tensor.matmul(
                        out=predp[32 * b:32 * b + 1, 0:NCH],
                        lhsT=wq[:, i, 0:1], rhs=ot[:, i, 1:9],
                        start=(i == 0), stop=(i == 5),
                        tile_position=(0, 32 * b))
                nc.vector.tensor_copy(out=pred_sb[32 * b:32 * b + 1, :],
                                      in_=predp[32 * b:32 * b + 1, 0:NCH])
                nc.sync.dma_start(out=out_d[b:b + 1, :],
                                  in_=pred_sb[32 * b:32 * b + 1, :])
                yield

            pending = None
            for b in range(BL):
                E1b = ep.tile([128, NCHUNK, EMB], bf16, tag="E1", name="E1b")
                E2b = ep.tile([128, NCHUNK, EMB], bf16, tag="E2", name="E2b")
                E0Tb = ep.tile([128, 6, NMEM], bf16, tag="E0T", name="E0Tb")
                E1Tb = ep.tile([128, 6, NMEM], bf16, tag="E1T", name="E1Tb")
                S0 = hps2.tile([128, NMEM], f32, tag="S", name="S0")
                for c in range(NCHUNK):
                    for tau in range(3):
                        tb = b * 3 + tau
                        g = gp.tile([128, 8, EMB], bf16, tag="g", name="g")
                        nc.gpsimd.dma_gather(
                            g[:], tab_d[:],
                            idx_sb[:, tb, 64 * c:64 * c + 64],
                            1024, 1024, EMB,
                            queue_num=(c * 3 + tau) % 2)
                        ps_a = wsps.tile([128, 512], f32, tag="wsa",
                                         name="ps_a")
                        ps_b = wsps.tile([128, 256], f32, tag="wsb",
                                         name="ps_b")
                        for gi in range(8):
                            k = gi // 2
                            lhs = sel_sb[:, 128 * gi + 32 * k:
                                         128 * gi + 32 * k + 32]
                            nc.tensor.matmul(
                                out=ps_a[32 * k:32 * k + 32, :], lhsT=lhs,
                                rhs=g[:, gi, 0:512],
                                start=(gi % 2 == 0), stop=(gi % 2 == 1),
                                tile_position=(0, 32 * k))
                            nc.tensor.matmul(
                                out=ps_b[32 * k:32 * k + 32, :], lhsT=lhs,
                                rhs=g[:, gi, 512:768],
                                start=(gi % 2 == 0), stop=(gi % 2 == 1),
                                tile_position=(0, 32 * k))
                        dl = 256 * tau
                        nc.any.tensor_copy(out=E1b[:, c, dl:dl + 256],
                                           in_=ps_a[:, 256:512])
                        nc.any.tensor_copy(out=E2b[:, c, dl:dl + 256],
                                           in_=ps_b[:])
                        stg = stgp.tile([128, 512], bf16, tag="stg",
                                        name="stg")
                        nc.any.tensor_copy(out=stg[:], in_=ps_a[:])
                        for q in range(4):  # 0,1 -> E0T ; 2,3 -> E1T
                            tp = tpps.tile([128, 128], bf16, tag="tp",
                                           name="tp")
                            nc.tensor.transpose(
                                out=tp[:], in_=stg[:, 128 * q:128 * q + 128],
                                identity=identb[:])
                            dst = E0Tb if q < 2 else E1Tb
                            nc.any.tensor_copy(
                                out=dst[:, 2 * tau + (q % 2),
                                        128 * c:128 * c + 128],
                                in_=tp[:])
                        if pending is not None and tau < 2:
                            next(pending, None)
                    if b == 0 and c == 0:
                        emit_uvw_dma()
                        emit_init()
                    if pending is not None:
                        next(pending, None)
                    # hop0 score columns for this chunk
                    for j in range(6):
                        nc.tensor.matmul(
                            out=S0[32 * b:32 * b + 9,
                                   128 * c:128 * c + 128],
                            lhsT=ysb0[:, j, 9 * b:9 * b + 9],
                            rhs=E0Tb[:, j, 128 * c:128 * c + 128],
                            start=(j == 0), stop=(j == 5),
                            tile_position=(0, 32 * b))
                pending = hop_chain(b, S0, E1b, E2b, E1Tb)
            for _ in pending:
                pass

    nc.compile()
    _cache[nu_pad] = nc
    return nc


def _prepare(subjects, relations, objects, ques, answerChoices,
             A_tables, B_table, U, V, W):
    subjects = np.asarray(subjects).astype(np.int64)
    relations = np.asarray(relations).astype(np.int64)
    objects = np.asarray(objects).astype(np.int64)
    ques = np.asarray(ques).astype(np.int64)
    answerChoices = np.asarray(answerChoices).astype(np.int64)
    A_tables = np.asarray(A_tables, dtype=np.float32)
    B_table = np.asarray(B_table, dtype=np.float32)

    a_cat = np.concatenate([A_tables[0], A_tables[1], A_tables[2]],
                           axis=1).astype(BF)
    b_bf = B_table.astype(BF)
    ut = np.ascontiguousarray(np.asarray(U, dtype=np.float32).T).astype(BF)
    vt = np.ascontiguousarray(np.asarray(V, dtype=np.float32).T).astype(BF)
    w_bf = np.ascontiguousarray(np.asarray(W, dtype=np.float32)).astype(BF)
    identb = np.eye(128, dtype=BF)
    identf = np.eye(128, dtype=np.float32)
    maskq = np.zeros((128, 1), dtype=np.float32)
    maskq[0::32] = 1.0
    # fixed word-sum selection: slot s = gi*128+p -> mem 16*gi + p//8
    p = np.arange(128)
    sel = np.zeros((128, 8, 128), dtype=BF)
    for gi in range(8):
        sel[p, gi, 16 * gi + p // 8] = 1.0
    sel = sel.reshape(128, 8 * 128)
    # init placement matrices (state row = 32*b + tc)
    ones3 = np.zeros((3, 128, 128), dtype=BF)
    ones3[0, p, 32 * (p // 32)] = 1.0                        # u rows
    ones3[1, p, 32 * (p // 64) + 1 + (p // 8) % 8] = 1.0     # a, b in {0,1}
    ones3[2, p, 32 * (2 + p // 64) + 1 + (p // 8) % 8] = 1.0  # a, b in {2,3}

    toks = [subjects, relations, objects]
    uniqs, streams, buniqs, idxuas = [], [], [], []
    nu_max = 0
    for core in range(NCORES):
        sl = slice(core * BL, (core + 1) * BL)
        # stream order: b, tau, chunk, m_local, w
        allt = np.stack([t[sl] for t in toks], axis=1)  # [BL, 3, 512, 8]
        uniq, inv = np.unique(allt.reshape(-1), return_inverse=True)
        if len(uniq) > 32752:
            raise OverflowError(f"core {core}: {len(uniq)} unique tokens")
        uniqs.append(uniq)
        streams.append(inv.astype(np.int64))
        nu_max = max(nu_max, len(uniq))
        # b-table side
        bt = np.concatenate([ques[sl].reshape(-1),
                             answerChoices[sl].reshape(-1)])
        bu, binv = np.unique(bt, return_inverse=True)
        assert len(bu) <= BU
        buniqs.append(bu)
        qinv = binv[:BL * QLEN].reshape(BL, QLEN)
        ainv = binv[BL * QLEN:].reshape(BL, NCH, CLEN)
        idxua = np.zeros((128, 3), dtype=np.int32)
        idxua[:, 0] = qinv[p // 32, p % 32]
        idxua[:, 1] = ainv[p // 64, (p // 8) % 8, p % 8]
        idxua[:, 2] = ainv[2 + p // 64, (p // 8) % 8, p % 8]
        idxuas.append(idxua)
    nu_pad = -(-nu_max // 16) * 16

    nc = _build_program(nu_pad)

    in_maps = []
    for core in range(NCORES):
        tab = np.zeros((nu_pad, EMB), dtype=BF)
        tab[:len(uniqs[core])] = a_cat[uniqs[core]]
        btab = np.zeros((BU, EMB), dtype=BF)
        btab[:len(buniqs[core])] = b_bf[buniqs[core]]
        idx16 = np.zeros((128, 3 * BL, NCHUNK * 64), dtype=np.int16)
        stream = streams[core].reshape(BL, 3, NCHUNK, 1024)
        for b in range(BL):
            for tau in range(3):
                for c in range(NCHUNK):
                    idx16[:, b * 3 + tau, 64 * c:64 * c + 64] = \
                        _wrap_idx16(stream[b, tau, c])
        in_maps.append(dict(
            tab=tab, btab=btab, ut=ut, vt=vt, w=w_bf, idx16=idx16,
            sel=sel, ones3=ones3, maskq=maskq, identb=identb,
            identf=identf, idxua=idxuas[core]))
    return nc, in_maps


def kernel(subjects, relations, objects, ques, answerChoices,
           A_tables, B_table, U, V, W):
    nc, in_maps = _prepare(subjects, relations, objects, ques, answerChoices,
                           A_tables, B_table, U, V, W)
    res = run_bass_kernel_spmd(nc, in_maps, list(range(NCORES)))
    return np.concatenate([res.results[c]["pred"] for c in range(NCORES)],
                          axis=0).astype(np.float32)


def profile(subjects, relations, objects, ques, answerChoices,
            A_tables, B_table, U, V, W, tmpdir=None):
    import os, tempfile
    if tmpdir is None:
        tmpdir = tempfile.mkdtemp(prefix="ktrace_")
    os.makedirs(tmpdir, exist_ok=True)
    nc, in_maps = _prepare(subjects, relations, objects, ques, answerChoices,
                           A_tables, B_table, U, V, W)
    res = run_bass_kernel_spmd(nc, in_maps, list(range(NCORES)),
                               trace=True, tmpdir=tmpdir)
    print(f"trace dir: {tmpdir}")
    return res.exec_time_ns
